# revision 20
# baseline (speedup 1.0000x reference)
"""Trainium2 Bass kernel: 16-head attention (B=2, S=2048, D=1024), 8-way sharded.

Sharding: core c handles batch b = c//4 and heads [4*(c%4), 4*(c%4)+4).
Megatron-style: Wq/Wk/Wv column-sharded (256 rows each), Wo row-sharded
(256 columns each); per-core partial outputs are summed on host.

Per-core device program (matmul inputs in fp16, fp32 PSUM accumulation):
  qT = (Wq_s @ x_q.T) + bq_s          [256, 2048]   (heads on partitions)
  kT = (Wk_s @ x_k.T) + bk_s          [256, 2048]
  v  = x_v @ Wv_s.T                   [2048, 256]   (no bias; folded on host)
  per head pair, per 512-col q-chunk, streaming over 16 key tiles:
    ST = k_h q_h.T (row-tiled K=64 pair, concurrent in the PE array) ->
    exp(0.125*ST) on ACT -> ctxT_ext accumulate = [v_h | 1].T @ PT
    (denominator fused as extra row: even head ones at col 64, odd at col 0)
  normalize: stage psum to SBUF fp16, K=1 matmul pair broadcasts the two
  denominator rows into one [128,512] psum tile, reciprocal_approx_fast on
  DVE (keeps the reciprocal OFF the ACT engine, which paces the kernel with
  the exp stream), scalar_tensor_tensor multiply -> ctxT (fp16)
  out_partial = ctxT.T @ Wo_s.T       [2048, 1024]  (fp16 partials)
Scheduling: flat 128-step schedule; every (c,t) pair is split into sk 0-7
and sk 8-15 halves (all low halves first, partials staged to SBUF fp16 and
re-added at flush) so the k/v projection chunks for sk 8-15 inject into the
second phase where the ACT-paced loop has PE slack. Head = t0-halves of
k/q chunk 0 only; everything else (v waves, remaining k/q chunks, wo load,
flushes, out-projection groups) fires from a deferred queue, one item per
step. Critical head transfers split across 8 DMA rings (fair packet
interleave makes ring count the bandwidth weight); constants are memset on
device; dummy matmuls + a dummy exp during the DMA lead-in warm the PE
clock gate and preload the exp table. Tail out-projection evacuates via the
idle ACT engine with a 4-buffer psum rotation and split normalize pieces.
Host: out[b] = sum of 4 fp16 partials + bo + Wo @ bv.
"""

import sys

sys.path.insert(0, "/opt/trn_rl_repo")

import functools

import numpy as np

import concourse.bass as bass
import concourse.mybir as mybir
import concourse.tile as tile
from concourse.bass_utils import run_bass_kernel_spmd
from concourse.vector_clock import ScopedClock, VectorClock

P = 128
S = 2048
D = 1024
M = 256  # local head dims per core (4 heads x 64)
NSQ = 4  # 512-wide query chunks
NSK = 16  # 128-row key tiles
NDC = 8  # 128-row chunks of the model dim
F32 = mybir.dt.float32
F16 = mybir.dt.float16
EXP = mybir.ActivationFunctionType.Exp
MUL = mybir.AluOpType.mult
ADD = mybir.AluOpType.add


def _drain_and_barrier_split(self, tick_clock, wait_clock):
    # The stock tail emits one Drain carrying a sem wait per live processor;
    # this walrus build rejects >1 sync wait on an instruction. Emit one
    # Drain per processor instead, each carrying a single wait.
    nc = self.nc
    vclock = tick_clock.global_clock
    n = len(vclock)
    for i in range(n):
        t = vclock[i]
        if t > 0:
            vc = VectorClock([0] * n)
            vc.require_at_least(i, t)
            inst = nc.sync.drain()
            wait_clock.add_sem_waits(inst.ins, ScopedClock({None: vc}))
    nc.all_engine_barrier()
    assert self.sems is not None
    popped = nc._tile_sem_poison_stack.pop()
    assert popped is self._sem_poison
    nc.clear_and_free_semaphores(list(self.sems.allocated().values()))
    nc.all_engine_barrier()


tile.TileContext._drain_and_barrier = _drain_and_barrier_split


def _split_multi_waits(nc, cap=1):
    """This walrus build rejects instructions carrying more than one sync
    wait. Move surplus waits onto nop instructions inserted just before the
    offending instruction on the same engine (engine FIFO preserves order)."""
    import bass_rust

    n_nops = 0
    for f in nc.m.functions:
        for blk in f.blocks:
            insts = blk.instructions  # live list view
            i = 0
            while i < len(insts):
                inst = insts[i]
                si = inst.sync_info
                if si is not None and len(si.on_wait) > cap:
                    waits = list(si.on_wait)
                    extra, keep = waits[:-cap], waits[-cap:]
                    pos = i
                    for j in range(0, len(extra), cap):
                        chunk = extra[j : j + cap]
                        bi = nc.engines[inst.engine].nop(nofuse=True)
                        nop_inst = bi.ins
                        tail = nc.cur_bb.bb.instructions
                        assert tail[-1].name == nop_inst.name
                        tail.pop()
                        nop_inst.sync_info = bass_rust.SyncInfo(
                            on_wait=chunk, on_update=[]
                        )
                        insts.insert(pos, nop_inst)
                        pos += 1
                        i += 1
                        n_nops += 1
                    inst.sync_info = bass_rust.SyncInfo(
                        on_wait=keep, on_update=list(si.on_update)
                    )
                i += 1
    return n_nops


@functools.lru_cache(maxsize=1)
def _build():
    nc = bass.Bass()
    xq = nc.declare_dram_parameter("xq", [D, S], F16, isOutput=False)
    xk = nc.declare_dram_parameter("xk", [D, S], F16, isOutput=False)
    xv = nc.declare_dram_parameter("xv", [D, S], F16, isOutput=False)
    wq = nc.declare_dram_parameter("wq", [D, M], F16, isOutput=False)
    wk = nc.declare_dram_parameter("wk", [D, M], F16, isOutput=False)
    wv = nc.declare_dram_parameter("wv", [D, M], F16, isOutput=False)
    wo = nc.declare_dram_parameter("wo", [M, D], F16, isOutput=False)
    bqk = nc.declare_dram_parameter("bqk", [4 * P], F32, isOutput=False)
    out = nc.declare_dram_parameter("out", [S, D], F16, isOutput=True)

    xq_r = xq.rearrange("(dc p) s -> p dc s", p=P)
    xk_r = xk.rearrange("(dc p) s -> p dc s", p=P)
    xv_r = xv.rearrange("(dc p) s -> p dc s", p=P)

    with tile.TileContext(nc) as tc:
        with tc.tile_pool(name="res", bufs=1) as res:
            wq_sb = res.tile([P, NDC, M], F16, name="wq_sb")
            wk_sb = res.tile([P, NDC, M], F16, name="wk_sb")
            wv_sb = res.tile([P, NDC, M], F16, name="wv_sb")
            wo_sb = res.tile([P, 2, D], F16, name="wo_sb")
            bqk_sb = res.tile([P, 4], F32, name="bqk_sb")
            ones_sb = res.tile([P, P], F16, name="ones_sb")
            warm_sb = res.tile([P, 2], F32, name="warm_sb")
            qT_sb = res.tile([P, 2, S], F16, name="qT_sb")
            kT_sb = res.tile([P, 2, S], F16, name="kT_sb")
            ctxT_sb = res.tile([P, 2, S], F16, name="ctxT_sb")
            # per key-tile, per pair: [v_even | ones] (65 cols used) and a
            # 128-wide odd slab: col 0 = ones, cols 64:128 = v_odd dims.
            v_e = res.tile([P, NSK, 2, P], F16, name="v_e")
            v_o = res.tile([P, NSK, 2, P], F16, name="v_o")

            # constants on-device (the 0.5MB inline-constant uploads each
            # held the serial Sync DMA queue for ~4-5us)
            nc.vector.memset(ones_sb[:], 1.0)
            nc.vector.memset(warm_sb[:, 0:1], 0.0)
            nc.vector.memset(v_e[:, :, :, 64:65], 1.0)
            nc.vector.memset(v_o[:, :, :, 0:1], 1.0)
            nc.vector.memset(v_o[:, :, :, 1:64], 0.0)

            # preload the exp activation table set (~2.7us) under the DMA
            # lead-in instead of paying it on the first real score tile
            nc.scalar.activation(warm_sb[:, 1:2], warm_sb[:, 0:1], EXP)

            # wk first: the k-projection t0 half is the first real PE work
            wk_r = wk.rearrange("(c p) m -> p c m", p=P)
            nc.sync.dma_start(out=wk_sb[:, :, 0:P], in_=wk_r[:, :, 0:P])
            nc.sync.dma_start(out=wk_sb[:, :, P:M], in_=wk_r[:, :, P:M])
            nc.sync.dma_start(out=bqk_sb[:], in_=bqk.rearrange("(g p) -> p g", p=P))

            # single pool scope: head work draws psum from ps_b so the head
            # and attention phases share one PSUM budget (4+2+2 = 8 banks)
            with (
                tc.tile_pool(name="ptp", bufs=8) as ptp,
                tc.tile_pool(name="rcpp", bufs=12) as rcpp,
                tc.tile_pool(name="ps_s", bufs=2, space="PSUM") as ps_s,
                tc.tile_pool(name="ps_c", bufs=2, space="PSUM") as ps_c,
                tc.tile_pool(name="ps_b", bufs=2, space="PSUM") as ps_b,
                tc.tile_pool(name="osb", bufs=8) as osb,
                tc.tile_pool(name="xs2", bufs=6) as xs2,
            ):
                # dummy matmuls during the DMA lead-in: sustained PE activity
                # flips the HAM clock gate to 8/8 so the first projection
                # chunks run at 2.4GHz instead of 1.2GHz
                warm_ps = ps_b.tile([P, 512], F32, name="bc", tag="bc")
                for _ in range(32):
                    nc.tensor.matmul(
                        warm_ps[0:1, 0:P],
                        lhsT=ones_sb[0:1, 0:1],
                        rhs=ones_sb[0:1, :],
                        start=True,
                        stop=True,
                    )

                def head_t0(xin_r, w_sb, b_off, dst):
                    # head-pair-0 half of projection chunk 0: 8 per-dc DMAs
                    # (ring-count weighting: critical transfers get 8 of the
                    # active rings so fair packet interleave favors them)
                    xt = xs2.tile([P, NDC, 512], F16, name="xch2", tag="xch2")
                    for dc in range(NDC):
                        nc.sync.dma_start(
                            out=xt[:, dc, :], in_=xin_r[:, dc, 0:512]
                        )
                    ps0 = ps_b.tile([P, 512], F32, name="bc", tag="bc")
                    for dc in range(NDC):
                        nc.tensor.matmul(
                            ps0[:],
                            lhsT=w_sb[:, dc, 0:P],
                            rhs=xt[:, dc, :],
                            start=(dc == 0),
                            stop=(dc == NDC - 1),
                        )
                    nc.vector.tensor_scalar_add(
                        dst[:, 0, 0:512], ps0[:], bqk_sb[:, b_off : b_off + 1]
                    )
                    return xt

                def make_kq_t1(xt, w_sb, b_off, dst):
                    # deferred head-pair-1 half of chunk 0 (reuses the DMA'd
                    # x tile captured from the head phase)
                    def emit():
                        ps1 = ps_b.tile([P, 512], F32, name="bc", tag="bc")
                        for dc in range(NDC):
                            nc.tensor.matmul(
                                ps1[:],
                                lhsT=w_sb[:, dc, P : 2 * P],
                                rhs=xt[:, dc, :],
                                start=(dc == 0),
                                stop=(dc == NDC - 1),
                            )
                        nc.vector.tensor_scalar_add(
                            dst[:, 1, 0:512],
                            ps1[:],
                            bqk_sb[:, b_off + 1 : b_off + 2],
                        )

                    return emit
                def make_flush(t, cs, u, split=False):
                    # Deferred tail of the previous (t, c) iteration. K=1
                    # matmul pair broadcasts the two denominator rows into
                    # one psum bank (concurrent: disjoint row AND col groups)
                    # and reciprocal_approx_fast runs on DVE, keeping the
                    # reciprocal chain entirely off the ACT engine.
                    def flush():
                        pb = ps_b.tile([P, 512], F32, name="bc", tag="bc")
                        nc.tensor.matmul(
                            pb[0:64, :],
                            lhsT=ones_sb[64:65, 0:64],
                            rhs=u[64:65, 0:512],
                            start=True,
                            stop=True,
                        )
                        nc.tensor.matmul(
                            pb[64:P, :],
                            lhsT=ones_sb[0:1, 64:P],
                            rhs=u[0:1, 512:1024],
                            start=True,
                            stop=True,
                        )
                        rcp = rcpp.tile([P, 512], F32, name="rcp", tag="rcp")
                        nc.vector.reciprocal_approx_fast(out=rcp[:], in_=pb[:])
                        # the tail flush splits the normalize into 128-q
                        # pieces so the trailing out-projection items start
                        # as soon as their q-range is normalized
                        npc = 4 if split else 1
                        w = 512 // npc
                        for pc in range(npc):
                            ps = slice(pc * w, (pc + 1) * w)
                            cps = slice(cs.start + pc * w, cs.start + (pc + 1) * w)
                            nc.vector.scalar_tensor_tensor(
                                out=ctxT_sb[0:64, t, cps],
                                in0=rcp[0:64, ps],
                                scalar=1.0,
                                in1=u[0:64, ps],
                                op0=MUL,
                                op1=MUL,
                            )
                            nc.vector.scalar_tensor_tensor(
                                out=ctxT_sb[64:P, t, cps],
                                in0=rcp[64:P, ps],
                                scalar=1.0,
                                in1=u[64:P, slice(512 + pc * w, 512 + (pc + 1) * w)],
                                op0=MUL,
                                op1=MUL,
                            )

                    return flush

                def make_outproj(c):
                    # 8 psum-group emitters for the s-range of chunk c;
                    # consumed one per sk-step inside a later iteration so
                    # PE work fills ACT-bound bubbles. The last chunk runs
                    # after the exp stream ends: its items alternate psum
                    # pools (4-buffer rotation) and evacuate via the idle
                    # ACT engine so the tail chain isn't DVE/psum-bound.
                    items = []
                    tail = c == NSQ - 1
                    for i, (st, jc) in enumerate(
                        (st, jc)
                        for st in range(4 * c, 4 * c + 4)
                        for jc in range(2)
                    ):

                        def emit(st=st, jc=jc, alt=(tail and i % 2 == 1)):
                            if alt:
                                po_t = ps_s.tile([P, 1024], F32, name="scores")
                                po = po_t[:, 0:512]
                            else:
                                po = ps_b.tile([P, 512], F32, name="bc", tag="bc")[:]
                            for tt in range(2):
                                nc.tensor.matmul(
                                    po,
                                    lhsT=ctxT_sb[:, tt, st * P : (st + 1) * P],
                                    rhs=wo_sb[:, tt, jc * 512 : (jc + 1) * 512],
                                    start=(tt == 0),
                                    stop=(tt == 1),
                                )
                            ot = osb.tile([P, 512], F16, name="ot")
                            if tail:
                                nc.scalar.copy(ot[:], po)
                            else:
                                nc.vector.tensor_copy(ot[:], po)
                            nc.sync.dma_start(
                                out=out[
                                    st * P : (st + 1) * P,
                                    jc * 512 : (jc + 1) * 512,
                                ],
                                in_=ot[:],
                            )

                        items.append(emit)
                    return items

                def make_kqchunk(xin_r, w_sb, b_off, dst, c):
                    # deferred projection chunk (k chunks 1-3, q chunks 1-3),
                    # injected into attention iterations so the exp stream
                    # starts early and PE fills ACT-bound bubbles
                    def emit():
                        xt = xs2.tile([P, NDC, 512], F16, name="xch2", tag="xch2")
                        cs = slice(c * 512, (c + 1) * 512)
                        nc.sync.dma_start(out=xt[:], in_=xin_r[:, :, cs])
                        ps2 = [
                            ps_b.tile([P, 512], F32, name="bc", tag="bc")
                            for _ in range(2)
                        ]
                        for dc in range(NDC):
                            for tt in range(2):
                                nc.tensor.matmul(
                                    ps2[tt][:],
                                    lhsT=w_sb[:, dc, tt * P : (tt + 1) * P],
                                    rhs=xt[:, dc, :],
                                    start=(dc == 0),
                                    stop=(dc == NDC - 1),
                                )
                        for tt in range(2):
                            nc.vector.tensor_scalar_add(
                                dst[:, tt, c * 512 : (c + 1) * 512],
                                ps2[tt][:],
                                bqk_sb[:, b_off + tt : b_off + tt + 1],
                            )

                    return emit

                def make_vwave2(u2):
                    # deferred v projection for s-tiles {2*u2, 2*u2+1}; two
                    # live psum accumulators so it fits the ps_b pool
                    def emit():
                        xt = xs2.tile([P, NDC, 512], F16, name="xch2", tag="xch2")
                        us = slice(u2 * 256, (u2 + 1) * 256)
                        nc.sync.dma_start(out=xt[:, :, 0:256], in_=xv_r[:, :, us])
                        psvs = [
                            ps_b.tile([P, 512], F32, name="bc", tag="bc")
                            for _ in range(2)
                        ]
                        for dc in range(NDC):
                            for j in range(2):
                                nc.tensor.matmul(
                                    psvs[j][:, 0:M],
                                    lhsT=xt[:, dc, j * P : (j + 1) * P],
                                    rhs=wv_sb[:, dc, :],
                                    start=(dc == 0),
                                    stop=(dc == NDC - 1),
                                )
                        for j in range(2):
                            st = 2 * u2 + j
                            psv_r = psvs[j][:, 0:M].rearrange(
                                "p (t m) -> p t m", t=2
                            )
                            nc.vector.tensor_copy(
                                v_e[:, st, :, 0:64], psv_r[:, :, 0:64]
                            )
                            nc.vector.tensor_copy(
                                v_o[:, st, :, 64:P], psv_r[:, :, 64:P]
                            )

                    return emit

                def wo_dma():
                    nc.sync.dma_start(
                        out=wo_sb[:], in_=wo.rearrange("(t p) j -> p t j", p=P)
                    )

                # ---- head: t0 halves of chunk 0 only ----
                xt_k = head_t0(xk_r, wk_sb, 2, kT_sb)
                wq_r = wq.rearrange("(c p) m -> p c m", p=P)
                nc.sync.dma_start(out=wq_sb[:, :, 0:P], in_=wq_r[:, :, 0:P])
                nc.sync.dma_start(out=wq_sb[:, :, P:M], in_=wq_r[:, :, P:M])
                xt_q = head_t0(xq_r, wq_sb, 0, qT_sb)
                nc.sync.dma_start(
                    out=wv_sb[:], in_=wv.rearrange("(c p) m -> p c m", p=P)
                )

                # ---- flat 128-step attention schedule ----
                # Every (c,t) pair is split into a low half (sk 0-7) and a
                # high half (sk 8-15); all low halves run first. The split
                # halves the ctx-psum accumulation (partials staged to SBUF
                # fp16 and summed on DVE at flush time) and moves the k/v
                # projection deadlines for sk 8-15 into the second phase, so
                # the deferred projection work spreads over 2x the exp-stream
                # time instead of piling into the first iteration.
                ct8 = [(c, t) for c in range(NSQ) for t in range(2)]
                sched = [(c, t, sk) for (c, t) in ct8 for sk in range(8)]
                sched += [(c, t, sk) for (c, t) in ct8 for sk in range(8, 16)]

                # [min_step, fn] items; one fires per step, first match in
                # list order. Deadlines: vw(n) covers ctx sk {2n, 2n+1}; k
                # chunk n covers score sk 4n..4n+3.
                deferred = [
                    [0, make_vwave2(0)],
                    [1, make_vwave2(1)],
                    [2, make_kqchunk(xk_r, wk_sb, 2, kT_sb, 1)],
                    [3, make_kq_t1(xt_k, wk_sb, 2, kT_sb)],
                    [4, make_vwave2(2)],
                    [5, make_kq_t1(xt_q, wq_sb, 0, qT_sb)],
                    [6, make_vwave2(3)],
                    [8, make_kqchunk(xq_r, wq_sb, 0, qT_sb, 1)],
                    [12, wo_dma],
                    [24, make_kqchunk(xq_r, wq_sb, 0, qT_sb, 2)],
                    [40, make_kqchunk(xq_r, wq_sb, 0, qT_sb, 3)],
                    [53, make_kqchunk(xk_r, wk_sb, 2, kT_sb, 2)],
                    [55, make_kqchunk(xk_r, wk_sb, 2, kT_sb, 3)],
                    [57, make_vwave2(4)],
                    [58, make_vwave2(5)],
                    [59, make_vwave2(6)],
                    [60, make_vwave2(7)],
                ]

                pcs = {}   # (c,t) -> [pc_e, pc_o]
                u_lo = {}  # (c,t) -> staged sk0-7 partial (fp16)

                def emit_ctx(g, c, t, sk, pt):
                    if (c, t) not in pcs:
                        pcs[(c, t)] = (
                            ps_c.tile([P, 512], F32, name="ctx", tag="ctx"),
                            ps_c.tile([P, 512], F32, name="ctx", tag="ctx"),
                        )
                    pc_e, pc_o = pcs[(c, t)]
                    nc.tensor.matmul(
                        pc_e[0:65, :],
                        lhsT=v_e[:, sk, t, 0:65],
                        rhs=pt[:, 0:512],
                        start=sk in (0, 8),
                        stop=sk in (7, 15),
                    )
                    nc.tensor.matmul(
                        pc_o[:, :],
                        lhsT=v_o[:, sk, t, :],
                        rhs=pt[:, 512:1024],
                        start=sk in (0, 8),
                        stop=sk in (7, 15),
                    )
                    if sk not in (7, 15):
                        return
                    # stage the finished half to SBUF fp16; frees the pair.
                    # The high half fuses the += of the staged low half into
                    # the evacuation via scalar_tensor_tensor.
                    u = rcpp.tile([P, 1024], F16, name="u", tag="u")
                    del pcs[(c, t)]
                    if sk == 7:
                        nc.vector.tensor_copy(u[0:65, 0:512], pc_e[0:65, :])
                        nc.vector.tensor_copy(u[:, 512:1024], pc_o[:, :])
                        u_lo[(c, t)] = u
                        return
                    ul = u_lo.pop((c, t))
                    nc.vector.scalar_tensor_tensor(
                        out=u[0:65, 0:512],
                        in0=pc_e[0:65, :],
                        scalar=1.0,
                        in1=ul[0:65, 0:512],
                        op0=MUL,
                        op1=ADD,
                    )
                    nc.vector.scalar_tensor_tensor(
                        out=u[:, 512:1024],
                        in0=pc_o[:, :],
                        scalar=1.0,
                        in1=ul[:, 512:1024],
                        op0=MUL,
                        op1=ADD,
                    )
                    cs = slice(c * 512, (c + 1) * 512)
                    tail_flush = c == NSQ - 1 and t == 1
                    deferred.append([g + 3, make_flush(t, cs, u, split=tail_flush)])
                    if t == 1:
                        for fi, fn in enumerate(make_outproj(c)):
                            deferred.append([g + 5 + fi, fn])

                LAG = 5
                lag = []
                for g, (c, t, sk) in enumerate(sched):
                    ks = slice(sk * P, (sk + 1) * P)
                    cs = slice(c * 512, (c + 1) * 512)
                    pss = ps_s.tile([P, 1024], F32, name="scores")
                    nc.tensor.matmul(
                        pss[:, 0:512],
                        lhsT=kT_sb[0:64, t, ks],
                        rhs=qT_sb[0:64, t, cs],
                        start=True,
                        stop=True,
                    )
                    nc.tensor.matmul(
                        pss[:, 512:1024],
                        lhsT=kT_sb[64:P, t, ks],
                        rhs=qT_sb[64:P, t, cs],
                        start=True,
                        stop=True,
                    )
                    pt = ptp.tile([P, 1024], F16, name="pt")
                    nc.scalar.activation(pt[:], pss[:], EXP, scale=0.125)
                    lag.append((c, t, sk, pt))
                    if len(lag) > LAG:
                        emit_ctx(g, *lag.pop(0))
                    for d in deferred:
                        if d[0] <= g:
                            d[1]()
                            deferred.remove(d)
                            break
                g = len(sched)
                while lag:
                    emit_ctx(g, *lag.pop(0))
                for d in list(deferred):
                    d[1]()

    # populate .instr bytes for InstISA subclasses (the custom-DVE
    # reciprocal); raw Bass skips this Bacc.compile() pass and the NEFF
    # compiler fails with "ISA wrong length" without it
    mybir.codegen_inst_isa_subclasses(nc)
    n = _split_multi_waits(nc)
    print(f"[kernel] split {n} multi-wait instructions into nops", flush=True)
    return nc


def _in_maps(query, key, value, Wq, bq, Wk, bk, Wv, Wo):
    maps = []
    for core in range(8):
        b = core // 4
        r0 = (core % 4) * M
        r1 = r0 + M
        maps.append(
            {
                "xq": np.ascontiguousarray(query[b].T).astype(np.float16),
                "xk": np.ascontiguousarray(key[b].T).astype(np.float16),
                "xv": np.ascontiguousarray(value[b].T).astype(np.float16),
                "wq": np.ascontiguousarray(Wq[r0:r1].T).astype(np.float16),
                "wk": np.ascontiguousarray(Wk[r0:r1].T).astype(np.float16),
                "wv": np.ascontiguousarray(Wv[r0:r1].T).astype(np.float16),
                "wo": np.ascontiguousarray(Wo[:, r0:r1].T).astype(np.float16),
                "bqk": np.ascontiguousarray(
                    np.concatenate([bq[r0:r1], bk[r0:r1]])
                ),
            }
        )
    return maps


def _gather(results, Wo, bv, bo):
    corr = (bo + Wo @ bv).astype(np.float32)
    full = np.empty((2, S, D), np.float32)
    for b in range(2):
        acc = results[4 * b]["out"].astype(np.float32)
        for i in range(1, 4):
            acc += results[4 * b + i]["out"].astype(np.float32)
        full[b] = acc + corr[None, :]
    return full


def kernel(query, key, value, Wq, bq, Wk, bk, Wv, bv, Wo, bo, _run_kwargs=None):
    query, key, value, Wq, bq, Wk, bk, Wv, bv, Wo, bo = (
        np.asarray(a, np.float32)
        for a in (query, key, value, Wq, bq, Wk, bk, Wv, bv, Wo, bo)
    )
    nc = _build()
    maps = _in_maps(query, key, value, Wq, bq, Wk, bk, Wv, Wo)
    res = run_bass_kernel_spmd(nc, maps, core_ids=list(range(8)), **(_run_kwargs or {}))
    out = _gather(res.results, Wo, bv, bo)
    if _run_kwargs:
        kernel.last_results = res
    return out
